# revision 13
# baseline (speedup 1.0000x reference)
"""Trainium2 Bass kernel for nn_EventADModel (2-layer event GRU + coord GRU + fusion MLP).

Strategy
--------
Pure data parallel across 8 NeuronCores: shard the B*T = 245760 (b,t) "tracks"
into 8 shards of 30720. All weights are replicated.

On-chip layout: hidden/gate dim on SBUF partitions, tracks on the free dim.
Host (numpy, free) pre-transposes inputs to [feature, N] and casts to bf16,
pre-collapses the fusion MLP (W1a@We, W1b@Wc), and applies b2 + valid mask to
the device output.  Step-1 GRU algebra (h==0) skips the hidden-state matmuls,
and (when biases are zero, which setup_inputs produces) the reset gate of
step-1 entirely.

Matmuls run in bf16 (fp32 PSUM accumulation); sigmoid/tanh on the scalar
engine; gate combining on the vector engine via fused scalar_tensor_tensor.
"""

import os
import sys

for _p in ("/opt/trn_rl_repo",):
    if os.path.isdir(_p) and _p not in sys.path:
        sys.path.insert(0, _p)

import numpy as np
import ml_dtypes

BF16 = ml_dtypes.bfloat16

# Problem constants (hardcoded per contract).
B, F, T, X = 8192, 2, 30, 64
HE, HC = 256, 32
N_CORES = 8
N_TOT = B * T              # 245760
NC_TRACKS = N_TOT // N_CORES  # 30720
NT = 256                   # tracks per main tile
SPAN = 512                 # tracks per coord-GRU span
G = 3 * HE                 # 768 gate rows

_CACHE = {}
LAST_RESULTS = None


def _pack_k(wT, k, m):
    """[k_tot, m] -> [128, (k_tot//128)*m] with K-chunks side by side."""
    kc = wT.shape[0] // 128
    return np.ascontiguousarray(
        wT.reshape(kc, 128, m).transpose(1, 0, 2).reshape(128, kc * m)
    )


def _build_program(zero_bias):
    import concourse.bacc as bacc
    import concourse.mybir as mybir
    from concourse import tile

    dt = mybir.dt
    AF = mybir.ActivationFunctionType
    OP = mybir.AluOpType

    nc = bacc.Bacc("TRN2", target_bir_lowering=False, debug=False,
                   num_devices=N_CORES)

    # ---- DRAM tensors -------------------------------------------------
    xt_d = nc.dram_tensor("xt", [128, NC_TRACKS], dt.bfloat16, kind="ExternalInput")
    ct_d = nc.dram_tensor("ct", [4, NC_TRACKS], dt.bfloat16, kind="ExternalInput")
    out_d = nc.dram_tensor("out", [NC_TRACKS, 2], dt.float32, kind="ExternalOutput")

    # Wih0.T duplicated on both partition halves so frame-0 matmuls use
    # rows 0:64 and frame-1 matmuls rows 64:128 (lhsT/rhs base must match).
    w0x_d = nc.dram_tensor("w0x", [128, G], dt.bfloat16, kind="ExternalInput")
    w0h_d = nc.dram_tensor("w0h", [128, 2 * G], dt.bfloat16, kind="ExternalInput")
    w1x_d = nc.dram_tensor("w1x", [128, 2 * G], dt.bfloat16, kind="ExternalInput")
    w1h_d = nc.dram_tensor("w1h", [128, 2 * G], dt.bfloat16, kind="ExternalInput")
    wc_d = nc.dram_tensor("wc", [4, 96], dt.bfloat16, kind="ExternalInput")
    wae_d = nc.dram_tensor("wae", [128, 256], dt.bfloat16, kind="ExternalInput")
    wac_d = nc.dram_tensor("wac", [32, 128], dt.bfloat16, kind="ExternalInput")
    w2t_d = nc.dram_tensor("w2t", [128, 2], dt.bfloat16, kind="ExternalInput")
    # biases packed as one [128, 18] f32: cols 0:4 rz0, 4:8 rz1, 8:10 bn0,
    # 10:12 bhn0, 12:14 bn1, 14:16 bhn1, 16 b_hid.
    bias_d = nc.dram_tensor("biases", [128, 18], dt.float32, kind="ExternalInput")
    biasc_d = nc.dram_tensor("biasc", [32, 8], dt.float32, kind="ExternalInput")

    TILES = NC_TRACKS // NT
    NSPANS = NC_TRACKS // SPAN

    with tile.TileContext(nc) as tc:
        with (
            tc.tile_pool(name="wpool", bufs=1) as wp,
            tc.tile_pool(name="xin", bufs=4) as xin,
            tc.tile_pool(name="cin", bufs=3) as cin,
            tc.tile_pool(name="gate", bufs=3) as gp,
            tc.tile_pool(name="state", bufs=2) as sp,
            tc.tile_pool(name="hcpool", bufs=1) as hcp,
            tc.tile_pool(name="outp", bufs=4) as op_,
            tc.tile_pool(name="gz", bufs=2, space="PSUM") as psg,
            tc.tile_pool(name="na", bufs=2, space="PSUM") as psn,
        ):
            # ---- resident weights ------------------------------------
            w0x = wp.tile([128, G], dt.bfloat16, name="w0x_s")
            w0h = wp.tile([128, 2 * G], dt.bfloat16, name="w0h_s")
            w1x = wp.tile([128, 2 * G], dt.bfloat16, name="w1x_s")
            w1h = wp.tile([128, 2 * G], dt.bfloat16, name="w1h_s")
            wc = wp.tile([4, 96], dt.bfloat16, name="wc_s")
            wae = wp.tile([128, 256], dt.bfloat16, name="wae_s")
            wac = wp.tile([32, 128], dt.bfloat16, name="wac_s")
            w2t = wp.tile([128, 2], dt.bfloat16, name="w2t_s")
            bias = wp.tile([128, 18], dt.float32, name="bias_s")
            biasc = wp.tile([32, 8], dt.float32, name="biasc_s")
            for sb_t, dr in ((w0x, w0x_d), (w0h, w0h_d), (w1x, w1x_d),
                             (w1h, w1h_d), (wc, wc_d), (wae, wae_d),
                             (wac, wac_d), (w2t, w2t_d), (bias, bias_d),
                             (biasc, biasc_d)):
                nc.sync.dma_start(sb_t[:], dr[:])

            hc_all = hcp.tile([32, NC_TRACKS], dt.bfloat16, name="hc_all")

            def sig_gates(g_ps, nchunks, bias_off, name):
                """sigmoid(g + bias) -> bf16 sbuf tile [128, nchunks*NT]."""
                outt = gp.tile([128, nchunks * NT], dt.bfloat16,
                               name=name, tag=name)
                if zero_bias:
                    nc.scalar.activation(outt[:], g_ps[:, :nchunks * NT],
                                         AF.Sigmoid)
                else:
                    for j in range(nchunks):
                        nc.scalar.activation(
                            outt[:, j * NT:(j + 1) * NT],
                            g_ps[:, j * NT:(j + 1) * NT], AF.Sigmoid,
                            bias=bias[:, bias_off + j:bias_off + j + 1])
                return outt

            def first_step(wxap, rhs_x, kc_x, bias_rz_off, bias_n_off,
                           bias_hn_off, hname):
                """GRU step with h==0. wxap(kc, j) -> lhsT AP; rhs_x(kc) -> AP."""
                if zero_bias:
                    g = psg.tile([128, 2 * NT], dt.float32, name="gfz", tag="gz")
                    jlist = [(2, 0), (3, 1)]  # z chunks only
                else:
                    g = psg.tile([128, 4 * NT], dt.float32, name="gf", tag="gz")
                    jlist = [(0, 0), (1, 1), (2, 2), (3, 3)]  # r and z
                for j, jj in jlist:
                    for kc in range(kc_x):
                        nc.tensor.matmul(
                            g[:, jj * NT:(jj + 1) * NT], wxap(kc, j),
                            rhs_x(kc), start=(kc == 0), stop=(kc == kc_x - 1))
                gn = psn.tile([128, 2 * NT], dt.float32, name="gnf", tag="na")
                for jj, j in enumerate((4, 5)):
                    for kc in range(kc_x):
                        nc.tensor.matmul(
                            gn[:, jj * NT:(jj + 1) * NT], wxap(kc, j),
                            rhs_x(kc), start=(kc == 0), stop=(kc == kc_x - 1))
                h = sp.tile([128, 2 * NT], dt.bfloat16, name=hname, tag=hname)
                n_s = gp.tile([128, 2 * NT], dt.bfloat16, name="nsf", tag="ns")
                if zero_bias:
                    z_s = sig_gates(g, 2, 0, "zsf")
                    nc.scalar.activation(n_s[:], gn[:], AF.Tanh)
                    e = gp.tile([128, 2 * NT], dt.bfloat16, name="ef", tag="e")
                    nc.vector.tensor_mul(e[:], z_s[:], n_s[:])
                    nc.vector.tensor_sub(h[:], n_s[:], e[:])
                else:
                    rz_s = sig_gates(g, 4, bias_rz_off, "rzsf")
                    u = gp.tile([128, 2 * NT], dt.bfloat16, name="uf", tag="u")
                    for c in range(2):
                        sl = slice(c * NT, (c + 1) * NT)
                        t_c = gp.tile([128, NT], dt.bfloat16, name="tf", tag="t")
                        nc.vector.tensor_scalar_mul(
                            t_c[:], rz_s[:, sl],
                            bias[:, bias_hn_off + c:bias_hn_off + c + 1])
                        nc.vector.scalar_tensor_tensor(
                            u[:, sl], gn[:, sl],
                            bias[:, bias_n_off + c:bias_n_off + c + 1],
                            t_c[:], OP.add, OP.add)
                    nc.scalar.activation(n_s[:], u[:], AF.Tanh)
                    z_s = rz_s[:, 2 * NT:4 * NT]
                    e = gp.tile([128, 2 * NT], dt.bfloat16, name="ef", tag="e")
                    nc.vector.tensor_mul(e[:], z_s, n_s[:])
                    nc.vector.tensor_sub(h[:], n_s[:], e[:])
                return h

            def full_step(wxap, kc_x, rhs_x, wh, h_prev, bias_rz_off,
                          bias_n_off, bias_hn_off, hname):
                """General GRU step: gates = wx@x + wh@h_prev."""
                g = psg.tile([128, 4 * NT], dt.float32, name="gg", tag="gz")
                for jj in range(4):  # r0 r1 z0 z1
                    for kc in range(kc_x):
                        nc.tensor.matmul(
                            g[:, jj * NT:(jj + 1) * NT], wxap(kc, jj),
                            rhs_x(kc), start=(kc == 0), stop=False)
                    for kc in range(2):
                        nc.tensor.matmul(
                            g[:, jj * NT:(jj + 1) * NT],
                            wh[:, kc * G + jj * 128:kc * G + (jj + 1) * 128],
                            h_prev[:, kc * NT:(kc + 1) * NT],
                            start=False, stop=(kc == 1))
                gn = psn.tile([128, 4 * NT], dt.float32, name="gng", tag="na")
                for jj, j in enumerate((4, 5)):  # in0 in1
                    for kc in range(kc_x):
                        nc.tensor.matmul(
                            gn[:, jj * NT:(jj + 1) * NT], wxap(kc, j),
                            rhs_x(kc), start=(kc == 0), stop=(kc == kc_x - 1))
                for jj, j in enumerate((4, 5)):  # hn0 hn1
                    for kc in range(2):
                        nc.tensor.matmul(
                            gn[:, (2 + jj) * NT:(3 + jj) * NT],
                            wh[:, kc * G + j * 128:kc * G + (j + 1) * 128],
                            h_prev[:, kc * NT:(kc + 1) * NT],
                            start=(kc == 0), stop=(kc == 1))
                rz_s = sig_gates(g, 4, bias_rz_off, "rzs")
                t = gp.tile([128, 2 * NT], dt.bfloat16, name="tg", tag="t2")
                u = gp.tile([128, 2 * NT], dt.bfloat16, name="ug", tag="u2")
                if zero_bias:
                    nc.vector.tensor_mul(t[:], rz_s[:, 0:2 * NT],
                                         gn[:, 2 * NT:4 * NT])
                    nc.vector.tensor_add(u[:], t[:], gn[:, 0:2 * NT])
                else:
                    for c in range(2):
                        sl = slice(c * NT, (c + 1) * NT)
                        sl_hn = slice((2 + c) * NT, (3 + c) * NT)
                        nc.vector.scalar_tensor_tensor(
                            t[:, sl], gn[:, sl_hn],
                            bias[:, bias_hn_off + c:bias_hn_off + c + 1],
                            rz_s[:, sl], OP.add, OP.mult)
                        nc.vector.scalar_tensor_tensor(
                            u[:, sl], gn[:, sl],
                            bias[:, bias_n_off + c:bias_n_off + c + 1],
                            t[:, sl], OP.add, OP.add)
                n_s = gp.tile([128, 2 * NT], dt.bfloat16, name="nsg", tag="ns")
                nc.scalar.activation(n_s[:], u[:], AF.Tanh)
                h = sp.tile([128, 2 * NT], dt.bfloat16, name=hname, tag=hname)
                d = gp.tile([128, 2 * NT], dt.bfloat16, name="dg", tag="d")
                e = gp.tile([128, 2 * NT], dt.bfloat16, name="eg", tag="e2")
                nc.vector.tensor_sub(d[:], h_prev[:], n_s[:])
                nc.vector.tensor_mul(e[:], rz_s[:, 2 * NT:4 * NT], d[:])
                nc.vector.tensor_add(h[:], n_s[:], e[:])
                return h

            def coord_span(s):
                ctile = cin.tile([4, SPAN], dt.bfloat16, name="ctile", tag="ct")
                nc.sync.dma_start(ctile[:], ct_d[:, s * SPAN:(s + 1) * SPAN])
                cps = psg.tile([32, 2 * SPAN], dt.float32, name="cps", tag="gz")
                nc.tensor.matmul(cps[:, 0:SPAN], wc[:, 32:64], ctile[:],
                                 start=True, stop=True)
                nc.tensor.matmul(cps[:, SPAN:2 * SPAN], wc[:, 64:96], ctile[:],
                                 start=True, stop=True)
                z_s = cin.tile([32, SPAN], dt.bfloat16, name="czs", tag="czs")
                n_s = cin.tile([32, SPAN], dt.bfloat16, name="cns", tag="cns")
                if zero_bias:
                    nc.scalar.activation(z_s[:], cps[:, 0:SPAN], AF.Sigmoid)
                    nc.scalar.activation(n_s[:], cps[:, SPAN:2 * SPAN], AF.Tanh)
                else:
                    nc.scalar.activation(z_s[:], cps[:, 0:SPAN], AF.Sigmoid,
                                         bias=biasc[:, 1:2])
                    # r gate + n path with biases
                    rps = psn.tile([32, SPAN], dt.float32, name="rps", tag="na")
                    nc.tensor.matmul(rps[:], wc[:, 0:32], ctile[:],
                                     start=True, stop=True)
                    r_s = cin.tile([32, SPAN], dt.bfloat16, name="crs", tag="crs")
                    nc.scalar.activation(r_s[:], rps[:], AF.Sigmoid,
                                         bias=biasc[:, 0:1])
                    tcd = cin.tile([32, SPAN], dt.bfloat16, name="ctd", tag="ctd")
                    nc.vector.tensor_scalar_mul(tcd[:], r_s[:], biasc[:, 3:4])
                    ucd = cin.tile([32, SPAN], dt.bfloat16, name="cud", tag="cud")
                    nc.vector.scalar_tensor_tensor(
                        ucd[:], cps[:, SPAN:2 * SPAN], biasc[:, 2:3], tcd[:],
                        OP.add, OP.add)
                    nc.scalar.activation(n_s[:], ucd[:], AF.Tanh)
                ec = cin.tile([32, SPAN], dt.bfloat16, name="cec", tag="cec")
                nc.vector.tensor_mul(ec[:], z_s[:], n_s[:])
                nc.vector.tensor_sub(hc_all[:, s * SPAN:(s + 1) * SPAN],
                                     n_s[:], ec[:])

            def fusion(i, h1):
                hid_ps = psn.tile([128, NT], dt.float32, name="hid_ps", tag="na")
                nc.tensor.matmul(hid_ps[:], wae[:, 0:128], h1[:, 0:NT],
                                 start=True, stop=False)
                nc.tensor.matmul(hid_ps[:], wae[:, 128:256], h1[:, NT:2 * NT],
                                 start=False, stop=False)
                nc.tensor.matmul(hid_ps[:], wac[:],
                                 hc_all[:, i * NT:(i + 1) * NT],
                                 start=False, stop=True)
                hid = gp.tile([128, NT], dt.bfloat16, name="hid", tag="hid")
                if zero_bias:
                    nc.vector.tensor_scalar(hid[:], hid_ps[:], 0.0, None,
                                            OP.max)
                else:
                    nc.vector.tensor_scalar(hid[:], hid_ps[:],
                                            bias[:, 16:17], 0.0,
                                            OP.add, OP.max)
                ops = psn.tile([128, 4], dt.float32, name="ops", tag="na")
                for c in range(2):
                    nc.tensor.matmul(ops[:, c * 2:(c + 1) * 2],
                                     hid[:, c * 128:(c + 1) * 128], w2t[:],
                                     start=True, stop=True)
                outt = op_.tile([128, 4], dt.float32, name="outt", tag="outt")
                nc.vector.tensor_copy(outt[:], ops[:])
                dst = out_d[:].rearrange("(a p) k -> p a k", p=128)
                nc.sync.dma_start(dst[:, 2 * i:2 * i + 2, :],
                                  outt[:].rearrange("p (c k) -> p c k", k=2))

            # ---- main loop -------------------------------------------
            for i in range(TILES):
                if i % 2 == 0:
                    coord_span(i // 2)
                xt = xin.tile([128, NT], dt.bfloat16, name="xt_t", tag="xt")
                nc.sync.dma_start(xt[:], xt_d[:, i * NT:(i + 1) * NT])
                x0 = lambda kc: xt[0:64, :]
                x1 = lambda kc: xt[64:128, :]
                w0f0 = lambda kc, j: w0x[0:64, j * 128:(j + 1) * 128]
                w0f1 = lambda kc, j: w0x[64:128, j * 128:(j + 1) * 128]
                w1ap = lambda kc, j: w1x[:, kc * G + j * 128:kc * G + (j + 1) * 128]
                h0_1 = first_step(w0f0, x0, 1, 0, 8, 10, "h01")
                h1_1 = first_step(
                    w1ap, lambda kc: h0_1[:, kc * NT:(kc + 1) * NT], 2,
                    4, 12, 14, "h11")
                h0_2 = full_step(w0f1, 1, x1, w0h, h0_1, 0, 8, 10, "h02")
                h1_2 = full_step(
                    w1ap, 2, lambda kc: h0_2[:, kc * NT:(kc + 1) * NT],
                    w1h, h1_1, 4, 12, 14, "h12")
                fusion(i, h1_2)

    nc.compile()
    return nc


def _prep_host(inputs):
    f32 = np.float32
    bf = np.asarray(inputs["batch_features"], dtype=f32)
    coords = np.asarray(inputs["coords"], dtype=f32)
    w = {k: np.asarray(inputs[k], dtype=f32) for k in inputs
         if k not in ("batch_features", "coords", "valid_mask")}

    XT = bf.transpose(1, 3, 0, 2).reshape(128, N_TOT)
    CT = coords.transpose(2, 0, 1).reshape(4, N_TOT)

    W1a, W1b = w["W1"][:, :128], w["W1"][:, 128:]
    wae = _pack_k(np.ascontiguousarray((W1a @ w["We"]).T), 256, 128)
    wac = np.ascontiguousarray((W1b @ w["Wc"]).T)
    b_hid = W1a @ w["be"] + W1b @ w["bc"] + w["b1"]

    def rzcols(b):  # (bih+bhh)[0:512] -> [128,4] cols r0 r1 z0 z1
        return np.ascontiguousarray(b[0:2 * HE].reshape(4, 128).T)

    bias = np.zeros((128, 18), f32)
    bias[:, 0:4] = rzcols(w["bih0"] + w["bhh0"])
    bias[:, 4:8] = rzcols(w["bih1"] + w["bhh1"])
    bias[:, 8:10] = w["bih0"][2 * HE:].reshape(2, 128).T
    bias[:, 10:12] = w["bhh0"][2 * HE:].reshape(2, 128).T
    bias[:, 12:14] = w["bih1"][2 * HE:].reshape(2, 128).T
    bias[:, 14:16] = w["bhh1"][2 * HE:].reshape(2, 128).T
    bias[:, 16] = b_hid
    biasc = np.zeros((32, 8), f32)
    bc_sum = w["bihC"] + w["bhhC"]
    biasc[:, 0] = bc_sum[0:HC]
    biasc[:, 1] = bc_sum[HC:2 * HC]
    biasc[:, 2] = w["bihC"][2 * HC:]
    biasc[:, 3] = w["bhhC"][2 * HC:]

    zero_bias = all(
        not np.any(w[k]) for k in
        ("bih0", "bhh0", "bih1", "bhh1", "bihC", "bhhC", "be", "bc", "b1"))

    wd = {
        "w0x": np.ascontiguousarray(
            np.concatenate([w["Wih0"].T, w["Wih0"].T], axis=0)).astype(BF16),
        "w0h": _pack_k(np.ascontiguousarray(w["Whh0"].T), 256, G).astype(BF16),
        "w1x": _pack_k(np.ascontiguousarray(w["Wih1"].T), 256, G).astype(BF16),
        "w1h": _pack_k(np.ascontiguousarray(w["Whh1"].T), 256, G).astype(BF16),
        "wc": np.ascontiguousarray(w["WihC"].T).astype(BF16),
        "wae": wae.astype(BF16),
        "wac": wac.astype(BF16),
        "w2t": np.ascontiguousarray(w["W2"].T).astype(BF16),
        "biases": bias,
        "biasc": biasc,
    }
    return XT, CT, wd, zero_bias, w["b2"]


def kernel(**inputs):
    global LAST_RESULTS
    from concourse.bass_utils import run_bass_kernel_spmd

    XT, CT, wd, zero_bias, b2 = _prep_host(inputs)

    key = ("v1", zero_bias)
    if key not in _CACHE:
        _CACHE[key] = _build_program(zero_bias)
    nc = _CACHE[key]

    in_maps = []
    for c in range(N_CORES):
        sl = slice(c * NC_TRACKS, (c + 1) * NC_TRACKS)
        m = dict(wd)
        m["xt"] = np.ascontiguousarray(XT[:, sl]).astype(BF16)
        m["ct"] = np.ascontiguousarray(CT[:, sl]).astype(BF16)
        in_maps.append(m)

    res = run_bass_kernel_spmd(nc, in_maps, list(range(N_CORES)))
    LAST_RESULTS = res

    out = np.concatenate([res.results[c]["out"] for c in range(N_CORES)], axis=0)
    out = out + b2[None, :].astype(np.float32)
    out = out.reshape(B, T, 2)
    mask = np.asarray(inputs["valid_mask"])
    return np.where(mask[:, :, None], out, np.float32(0.0)).astype(np.float32)


# revision 14
# speedup vs baseline: 1.0001x; 1.0001x over previous
"""Trainium2 Bass kernel for nn_EventADModel (2-layer event GRU + coord GRU + fusion MLP).

Strategy
--------
Pure data parallel across 8 NeuronCores: shard the B*T = 245760 (b,t) "tracks"
into 8 shards of 30720. All weights are replicated.

On-chip layout: hidden/gate dim on SBUF partitions, tracks on the free dim.
Host (numpy, free) pre-transposes inputs to [feature, N] and casts to bf16,
pre-collapses the fusion MLP (W1a@We, W1b@Wc), and applies b2 + valid mask to
the device output.  Step-1 GRU algebra (h==0) skips the hidden-state matmuls,
and (when biases are zero, which setup_inputs produces) the reset gate of
step-1 entirely.

Matmuls run in bf16 (fp32 PSUM accumulation); sigmoid/tanh on the scalar
engine; gate combining on the vector engine via fused scalar_tensor_tensor.
"""

import os
import sys

for _p in ("/opt/trn_rl_repo",):
    if os.path.isdir(_p) and _p not in sys.path:
        sys.path.insert(0, _p)

import numpy as np
import ml_dtypes

BF16 = np.float16

# Problem constants (hardcoded per contract).
B, F, T, X = 8192, 2, 30, 64
HE, HC = 256, 32
N_CORES = 8
N_TOT = B * T              # 245760
NC_TRACKS = N_TOT // N_CORES  # 30720
NT = 256                   # tracks per main tile
SPAN = 512                 # tracks per coord-GRU span
G = 3 * HE                 # 768 gate rows

_CACHE = {}
LAST_RESULTS = None


def _pack_k(wT, k, m):
    """[k_tot, m] -> [128, (k_tot//128)*m] with K-chunks side by side."""
    kc = wT.shape[0] // 128
    return np.ascontiguousarray(
        wT.reshape(kc, 128, m).transpose(1, 0, 2).reshape(128, kc * m)
    )


def _build_program(zero_bias):
    import concourse.bacc as bacc
    import concourse.mybir as mybir
    from concourse import tile

    dt = mybir.dt
    AF = mybir.ActivationFunctionType
    OP = mybir.AluOpType

    nc = bacc.Bacc("TRN2", target_bir_lowering=False, debug=False,
                   num_devices=N_CORES)

    # ---- DRAM tensors -------------------------------------------------
    xt_d = nc.dram_tensor("xt", [128, NC_TRACKS], dt.float16, kind="ExternalInput")
    ct_d = nc.dram_tensor("ct", [4, NC_TRACKS], dt.float16, kind="ExternalInput")
    out_d = nc.dram_tensor("out", [NC_TRACKS, 2], dt.float32, kind="ExternalOutput")

    # Wih0.T duplicated on both partition halves so frame-0 matmuls use
    # rows 0:64 and frame-1 matmuls rows 64:128 (lhsT/rhs base must match).
    w0x_d = nc.dram_tensor("w0x", [128, G], dt.float16, kind="ExternalInput")
    w0h_d = nc.dram_tensor("w0h", [128, 2 * G], dt.float16, kind="ExternalInput")
    w1x_d = nc.dram_tensor("w1x", [128, 2 * G], dt.float16, kind="ExternalInput")
    w1h_d = nc.dram_tensor("w1h", [128, 2 * G], dt.float16, kind="ExternalInput")
    wc_d = nc.dram_tensor("wc", [4, 96], dt.float16, kind="ExternalInput")
    wae_d = nc.dram_tensor("wae", [128, 256], dt.float16, kind="ExternalInput")
    wac_d = nc.dram_tensor("wac", [32, 128], dt.float16, kind="ExternalInput")
    w2t_d = nc.dram_tensor("w2t", [128, 2], dt.float16, kind="ExternalInput")
    # biases packed as one [128, 18] f32: cols 0:4 rz0, 4:8 rz1, 8:10 bn0,
    # 10:12 bhn0, 12:14 bn1, 14:16 bhn1, 16 b_hid.
    bias_d = nc.dram_tensor("biases", [128, 18], dt.float32, kind="ExternalInput")
    biasc_d = nc.dram_tensor("biasc", [32, 8], dt.float32, kind="ExternalInput")

    TILES = NC_TRACKS // NT
    NSPANS = NC_TRACKS // SPAN

    with tile.TileContext(nc) as tc:
        with (
            tc.tile_pool(name="wpool", bufs=1) as wp,
            tc.tile_pool(name="xin", bufs=4) as xin,
            tc.tile_pool(name="cin", bufs=3) as cin,
            tc.tile_pool(name="gate", bufs=3) as gp,
            tc.tile_pool(name="state", bufs=2) as sp,
            tc.tile_pool(name="hcpool", bufs=1) as hcp,
            tc.tile_pool(name="outp", bufs=4) as op_,
            tc.tile_pool(name="gz", bufs=2, space="PSUM") as psg,
            tc.tile_pool(name="na", bufs=2, space="PSUM") as psn,
        ):
            # ---- resident weights ------------------------------------
            w0x = wp.tile([128, G], dt.float16, name="w0x_s")
            w0h = wp.tile([128, 2 * G], dt.float16, name="w0h_s")
            w1x = wp.tile([128, 2 * G], dt.float16, name="w1x_s")
            w1h = wp.tile([128, 2 * G], dt.float16, name="w1h_s")
            wc = wp.tile([4, 96], dt.float16, name="wc_s")
            wae = wp.tile([128, 256], dt.float16, name="wae_s")
            wac = wp.tile([32, 128], dt.float16, name="wac_s")
            w2t = wp.tile([128, 2], dt.float16, name="w2t_s")
            bias = wp.tile([128, 18], dt.float32, name="bias_s")
            biasc = wp.tile([32, 8], dt.float32, name="biasc_s")
            for sb_t, dr in ((w0x, w0x_d), (w0h, w0h_d), (w1x, w1x_d),
                             (w1h, w1h_d), (wc, wc_d), (wae, wae_d),
                             (wac, wac_d), (w2t, w2t_d), (bias, bias_d),
                             (biasc, biasc_d)):
                nc.sync.dma_start(sb_t[:], dr[:])

            hc_all = hcp.tile([32, NC_TRACKS], dt.float16, name="hc_all")

            def sig_gates(g_ps, nchunks, bias_off, name):
                """sigmoid(g + bias) -> bf16 sbuf tile [128, nchunks*NT]."""
                outt = gp.tile([128, nchunks * NT], dt.float16,
                               name=name, tag=name)
                if zero_bias:
                    nc.scalar.activation(outt[:], g_ps[:, :nchunks * NT],
                                         AF.Sigmoid)
                else:
                    for j in range(nchunks):
                        nc.scalar.activation(
                            outt[:, j * NT:(j + 1) * NT],
                            g_ps[:, j * NT:(j + 1) * NT], AF.Sigmoid,
                            bias=bias[:, bias_off + j:bias_off + j + 1])
                return outt

            def first_step(wxap, rhs_x, kc_x, bias_rz_off, bias_n_off,
                           bias_hn_off, hname):
                """GRU step with h==0. wxap(kc, j) -> lhsT AP; rhs_x(kc) -> AP."""
                if zero_bias:
                    g = psg.tile([128, 2 * NT], dt.float32, name="gfz", tag="gz")
                    jlist = [(2, 0), (3, 1)]  # z chunks only
                else:
                    g = psg.tile([128, 4 * NT], dt.float32, name="gf", tag="gz")
                    jlist = [(0, 0), (1, 1), (2, 2), (3, 3)]  # r and z
                for j, jj in jlist:
                    for kc in range(kc_x):
                        nc.tensor.matmul(
                            g[:, jj * NT:(jj + 1) * NT], wxap(kc, j),
                            rhs_x(kc), start=(kc == 0), stop=(kc == kc_x - 1))
                gn = psn.tile([128, 2 * NT], dt.float32, name="gnf", tag="na")
                for jj, j in enumerate((4, 5)):
                    for kc in range(kc_x):
                        nc.tensor.matmul(
                            gn[:, jj * NT:(jj + 1) * NT], wxap(kc, j),
                            rhs_x(kc), start=(kc == 0), stop=(kc == kc_x - 1))
                h = sp.tile([128, 2 * NT], dt.float16, name=hname, tag=hname)
                n_s = gp.tile([128, 2 * NT], dt.float16, name="nsf", tag="ns")
                if zero_bias:
                    z_s = sig_gates(g, 2, 0, "zsf")
                    nc.scalar.activation(n_s[:], gn[:], AF.Tanh)
                    e = gp.tile([128, 2 * NT], dt.float16, name="ef", tag="e")
                    nc.vector.tensor_mul(e[:], z_s[:], n_s[:])
                    nc.vector.tensor_sub(h[:], n_s[:], e[:])
                else:
                    rz_s = sig_gates(g, 4, bias_rz_off, "rzsf")
                    u = gp.tile([128, 2 * NT], dt.float16, name="uf", tag="u")
                    for c in range(2):
                        sl = slice(c * NT, (c + 1) * NT)
                        t_c = gp.tile([128, NT], dt.float16, name="tf", tag="t")
                        nc.vector.tensor_scalar_mul(
                            t_c[:], rz_s[:, sl],
                            bias[:, bias_hn_off + c:bias_hn_off + c + 1])
                        nc.vector.scalar_tensor_tensor(
                            u[:, sl], gn[:, sl],
                            bias[:, bias_n_off + c:bias_n_off + c + 1],
                            t_c[:], OP.add, OP.add)
                    nc.scalar.activation(n_s[:], u[:], AF.Tanh)
                    z_s = rz_s[:, 2 * NT:4 * NT]
                    e = gp.tile([128, 2 * NT], dt.float16, name="ef", tag="e")
                    nc.vector.tensor_mul(e[:], z_s, n_s[:])
                    nc.vector.tensor_sub(h[:], n_s[:], e[:])
                return h

            def full_step(wxap, kc_x, rhs_x, wh, h_prev, bias_rz_off,
                          bias_n_off, bias_hn_off, hname):
                """General GRU step: gates = wx@x + wh@h_prev."""
                g = psg.tile([128, 4 * NT], dt.float32, name="gg", tag="gz")
                for jj in range(4):  # r0 r1 z0 z1
                    for kc in range(kc_x):
                        nc.tensor.matmul(
                            g[:, jj * NT:(jj + 1) * NT], wxap(kc, jj),
                            rhs_x(kc), start=(kc == 0), stop=False)
                    for kc in range(2):
                        nc.tensor.matmul(
                            g[:, jj * NT:(jj + 1) * NT],
                            wh[:, kc * G + jj * 128:kc * G + (jj + 1) * 128],
                            h_prev[:, kc * NT:(kc + 1) * NT],
                            start=False, stop=(kc == 1))
                gn = psn.tile([128, 4 * NT], dt.float32, name="gng", tag="na")
                for jj, j in enumerate((4, 5)):  # in0 in1
                    for kc in range(kc_x):
                        nc.tensor.matmul(
                            gn[:, jj * NT:(jj + 1) * NT], wxap(kc, j),
                            rhs_x(kc), start=(kc == 0), stop=(kc == kc_x - 1))
                for jj, j in enumerate((4, 5)):  # hn0 hn1
                    for kc in range(2):
                        nc.tensor.matmul(
                            gn[:, (2 + jj) * NT:(3 + jj) * NT],
                            wh[:, kc * G + j * 128:kc * G + (j + 1) * 128],
                            h_prev[:, kc * NT:(kc + 1) * NT],
                            start=(kc == 0), stop=(kc == 1))
                rz_s = sig_gates(g, 4, bias_rz_off, "rzs")
                t = gp.tile([128, 2 * NT], dt.float16, name="tg", tag="t2")
                u = gp.tile([128, 2 * NT], dt.float16, name="ug", tag="u2")
                if zero_bias:
                    nc.vector.tensor_mul(t[:], rz_s[:, 0:2 * NT],
                                         gn[:, 2 * NT:4 * NT])
                    nc.vector.tensor_add(u[:], t[:], gn[:, 0:2 * NT])
                else:
                    for c in range(2):
                        sl = slice(c * NT, (c + 1) * NT)
                        sl_hn = slice((2 + c) * NT, (3 + c) * NT)
                        nc.vector.scalar_tensor_tensor(
                            t[:, sl], gn[:, sl_hn],
                            bias[:, bias_hn_off + c:bias_hn_off + c + 1],
                            rz_s[:, sl], OP.add, OP.mult)
                        nc.vector.scalar_tensor_tensor(
                            u[:, sl], gn[:, sl],
                            bias[:, bias_n_off + c:bias_n_off + c + 1],
                            t[:, sl], OP.add, OP.add)
                n_s = gp.tile([128, 2 * NT], dt.float16, name="nsg", tag="ns")
                nc.scalar.activation(n_s[:], u[:], AF.Tanh)
                h = sp.tile([128, 2 * NT], dt.float16, name=hname, tag=hname)
                d = gp.tile([128, 2 * NT], dt.float16, name="dg", tag="d")
                e = gp.tile([128, 2 * NT], dt.float16, name="eg", tag="e2")
                nc.vector.tensor_sub(d[:], h_prev[:], n_s[:])
                nc.vector.tensor_mul(e[:], rz_s[:, 2 * NT:4 * NT], d[:])
                nc.vector.tensor_add(h[:], n_s[:], e[:])
                return h

            def coord_span(s):
                ctile = cin.tile([4, SPAN], dt.float16, name="ctile", tag="ct")
                nc.sync.dma_start(ctile[:], ct_d[:, s * SPAN:(s + 1) * SPAN])
                cps = psg.tile([32, 2 * SPAN], dt.float32, name="cps", tag="gz")
                nc.tensor.matmul(cps[:, 0:SPAN], wc[:, 32:64], ctile[:],
                                 start=True, stop=True)
                nc.tensor.matmul(cps[:, SPAN:2 * SPAN], wc[:, 64:96], ctile[:],
                                 start=True, stop=True)
                z_s = cin.tile([32, SPAN], dt.float16, name="czs", tag="czs")
                n_s = cin.tile([32, SPAN], dt.float16, name="cns", tag="cns")
                if zero_bias:
                    nc.scalar.activation(z_s[:], cps[:, 0:SPAN], AF.Sigmoid)
                    nc.scalar.activation(n_s[:], cps[:, SPAN:2 * SPAN], AF.Tanh)
                else:
                    nc.scalar.activation(z_s[:], cps[:, 0:SPAN], AF.Sigmoid,
                                         bias=biasc[:, 1:2])
                    # r gate + n path with biases
                    rps = psn.tile([32, SPAN], dt.float32, name="rps", tag="na")
                    nc.tensor.matmul(rps[:], wc[:, 0:32], ctile[:],
                                     start=True, stop=True)
                    r_s = cin.tile([32, SPAN], dt.float16, name="crs", tag="crs")
                    nc.scalar.activation(r_s[:], rps[:], AF.Sigmoid,
                                         bias=biasc[:, 0:1])
                    tcd = cin.tile([32, SPAN], dt.float16, name="ctd", tag="ctd")
                    nc.vector.tensor_scalar_mul(tcd[:], r_s[:], biasc[:, 3:4])
                    ucd = cin.tile([32, SPAN], dt.float16, name="cud", tag="cud")
                    nc.vector.scalar_tensor_tensor(
                        ucd[:], cps[:, SPAN:2 * SPAN], biasc[:, 2:3], tcd[:],
                        OP.add, OP.add)
                    nc.scalar.activation(n_s[:], ucd[:], AF.Tanh)
                ec = cin.tile([32, SPAN], dt.float16, name="cec", tag="cec")
                nc.vector.tensor_mul(ec[:], z_s[:], n_s[:])
                nc.vector.tensor_sub(hc_all[:, s * SPAN:(s + 1) * SPAN],
                                     n_s[:], ec[:])

            def fusion(i, h1):
                hid_ps = psn.tile([128, NT], dt.float32, name="hid_ps", tag="na")
                nc.tensor.matmul(hid_ps[:], wae[:, 0:128], h1[:, 0:NT],
                                 start=True, stop=False)
                nc.tensor.matmul(hid_ps[:], wae[:, 128:256], h1[:, NT:2 * NT],
                                 start=False, stop=False)
                nc.tensor.matmul(hid_ps[:], wac[:],
                                 hc_all[:, i * NT:(i + 1) * NT],
                                 start=False, stop=True)
                hid = gp.tile([128, NT], dt.float16, name="hid", tag="hid")
                if zero_bias:
                    nc.vector.tensor_scalar(hid[:], hid_ps[:], 0.0, None,
                                            OP.max)
                else:
                    nc.vector.tensor_scalar(hid[:], hid_ps[:],
                                            bias[:, 16:17], 0.0,
                                            OP.add, OP.max)
                ops = psn.tile([128, 4], dt.float32, name="ops", tag="na")
                for c in range(2):
                    nc.tensor.matmul(ops[:, c * 2:(c + 1) * 2],
                                     hid[:, c * 128:(c + 1) * 128], w2t[:],
                                     start=True, stop=True)
                outt = op_.tile([128, 4], dt.float32, name="outt", tag="outt")
                nc.vector.tensor_copy(outt[:], ops[:])
                dst = out_d[:].rearrange("(a p) k -> p a k", p=128)
                nc.sync.dma_start(dst[:, 2 * i:2 * i + 2, :],
                                  outt[:].rearrange("p (c k) -> p c k", k=2))

            # ---- main loop -------------------------------------------
            for i in range(TILES):
                if i % 2 == 0:
                    coord_span(i // 2)
                xt = xin.tile([128, NT], dt.float16, name="xt_t", tag="xt")
                nc.sync.dma_start(xt[:], xt_d[:, i * NT:(i + 1) * NT])
                x0 = lambda kc: xt[0:64, :]
                x1 = lambda kc: xt[64:128, :]
                w0f0 = lambda kc, j: w0x[0:64, j * 128:(j + 1) * 128]
                w0f1 = lambda kc, j: w0x[64:128, j * 128:(j + 1) * 128]
                w1ap = lambda kc, j: w1x[:, kc * G + j * 128:kc * G + (j + 1) * 128]
                h0_1 = first_step(w0f0, x0, 1, 0, 8, 10, "h01")
                h1_1 = first_step(
                    w1ap, lambda kc: h0_1[:, kc * NT:(kc + 1) * NT], 2,
                    4, 12, 14, "h11")
                h0_2 = full_step(w0f1, 1, x1, w0h, h0_1, 0, 8, 10, "h02")
                h1_2 = full_step(
                    w1ap, 2, lambda kc: h0_2[:, kc * NT:(kc + 1) * NT],
                    w1h, h1_1, 4, 12, 14, "h12")
                fusion(i, h1_2)

    nc.compile()
    return nc


def _prep_host(inputs):
    f32 = np.float32
    bf = np.asarray(inputs["batch_features"], dtype=f32)
    coords = np.asarray(inputs["coords"], dtype=f32)
    w = {k: np.asarray(inputs[k], dtype=f32) for k in inputs
         if k not in ("batch_features", "coords", "valid_mask")}

    XT = bf.transpose(1, 3, 0, 2).reshape(128, N_TOT)
    CT = coords.transpose(2, 0, 1).reshape(4, N_TOT)

    W1a, W1b = w["W1"][:, :128], w["W1"][:, 128:]
    wae = _pack_k(np.ascontiguousarray((W1a @ w["We"]).T), 256, 128)
    wac = np.ascontiguousarray((W1b @ w["Wc"]).T)
    b_hid = W1a @ w["be"] + W1b @ w["bc"] + w["b1"]

    def rzcols(b):  # (bih+bhh)[0:512] -> [128,4] cols r0 r1 z0 z1
        return np.ascontiguousarray(b[0:2 * HE].reshape(4, 128).T)

    bias = np.zeros((128, 18), f32)
    bias[:, 0:4] = rzcols(w["bih0"] + w["bhh0"])
    bias[:, 4:8] = rzcols(w["bih1"] + w["bhh1"])
    bias[:, 8:10] = w["bih0"][2 * HE:].reshape(2, 128).T
    bias[:, 10:12] = w["bhh0"][2 * HE:].reshape(2, 128).T
    bias[:, 12:14] = w["bih1"][2 * HE:].reshape(2, 128).T
    bias[:, 14:16] = w["bhh1"][2 * HE:].reshape(2, 128).T
    bias[:, 16] = b_hid
    biasc = np.zeros((32, 8), f32)
    bc_sum = w["bihC"] + w["bhhC"]
    biasc[:, 0] = bc_sum[0:HC]
    biasc[:, 1] = bc_sum[HC:2 * HC]
    biasc[:, 2] = w["bihC"][2 * HC:]
    biasc[:, 3] = w["bhhC"][2 * HC:]

    zero_bias = all(
        not np.any(w[k]) for k in
        ("bih0", "bhh0", "bih1", "bhh1", "bihC", "bhhC", "be", "bc", "b1"))

    wd = {
        "w0x": np.ascontiguousarray(
            np.concatenate([w["Wih0"].T, w["Wih0"].T], axis=0)).astype(BF16),
        "w0h": _pack_k(np.ascontiguousarray(w["Whh0"].T), 256, G).astype(BF16),
        "w1x": _pack_k(np.ascontiguousarray(w["Wih1"].T), 256, G).astype(BF16),
        "w1h": _pack_k(np.ascontiguousarray(w["Whh1"].T), 256, G).astype(BF16),
        "wc": np.ascontiguousarray(w["WihC"].T).astype(BF16),
        "wae": wae.astype(BF16),
        "wac": wac.astype(BF16),
        "w2t": np.ascontiguousarray(w["W2"].T).astype(BF16),
        "biases": bias,
        "biasc": biasc,
    }
    return XT, CT, wd, zero_bias, w["b2"]


def kernel(**inputs):
    global LAST_RESULTS
    from concourse.bass_utils import run_bass_kernel_spmd

    XT, CT, wd, zero_bias, b2 = _prep_host(inputs)

    key = ("v1", zero_bias)
    if key not in _CACHE:
        _CACHE[key] = _build_program(zero_bias)
    nc = _CACHE[key]

    in_maps = []
    for c in range(N_CORES):
        sl = slice(c * NC_TRACKS, (c + 1) * NC_TRACKS)
        m = dict(wd)
        m["xt"] = np.ascontiguousarray(XT[:, sl]).astype(BF16)
        m["ct"] = np.ascontiguousarray(CT[:, sl]).astype(BF16)
        in_maps.append(m)

    res = run_bass_kernel_spmd(nc, in_maps, list(range(N_CORES)))
    LAST_RESULTS = res

    out = np.concatenate([res.results[c]["out"] for c in range(N_CORES)], axis=0)
    out = out + b2[None, :].astype(np.float32)
    out = out.reshape(B, T, 2)
    mask = np.asarray(inputs["valid_mask"])
    return np.where(mask[:, :, None], out, np.float32(0.0)).astype(np.float32)
